# revision 10
# baseline (speedup 1.0000x reference)
"""Conv2d(128->256, 3x3, VALID) + InstanceNorm2d(affine=False) + /2 on Trainium2.

Contract: kernel(**inputs) takes FULL inputs (x:[16,128,128,128] f32,
weight:[256,128,3,3] f32, bias:[256] f32) and returns the FULL output
[16,256,126,126] f32.

Strategy:
- Data-parallel over batch N=16 across 8 NeuronCores (2 images/core).
- Conv lowered to 9 accumulated matmuls per output tile: contract dim is
  C_in=128 (exactly the PE partition dim), stationary operand is the
  128x128 weight slice for one (kh,kw,co-chunk), moving operand is a
  3-row strip of the input image ([128, 3, 126] AP into the resident
  x tile). float32r matmuls run at 1 cycle/row (4x faster than fp32).
- Bias is skipped: InstanceNorm with affine=False makes a per-channel
  additive constant cancel exactly (shifts mean only).
- Stats fused: ACT evacuates PSUM->SBUF with accum_out giving sum(y);
  DVE tensor_tensor_reduce(y*y) gives sum(y^2). Normalization is a
  single DVE tensor_scalar (y*alpha - mean*alpha) in place, then DMA.
"""

import numpy as np

import concourse.bass as bass
import concourse.tile as tile
from concourse import mybir
from concourse.vector_clock import ScopedClock

N, C_IN, H, W = 16, 128, 128, 128
C_OUT, KH, KW = 256, 3, 3
HO, WO = 126, 126
N_CORES = 8
N_PER_CORE = N // N_CORES  # 2
RG = 3                     # output rows per row-group (matmul free dim = 3*126 = 378)
NCOL = RG * WO             # 378 fp32 <= 512 (one PSUM bank)
N_RG = HO // RG            # 42 row-groups per (image, chunk)
BG = 7                     # row-groups per PSUM bank-group (7 of 8 banks in flight)
N_BG = N_RG // BG          # 6
PIX = HO * WO              # 15876
EPS = 1e-5

F32 = mybir.dt.float32
F32R = mybir.dt.float32r


class _SplitDrainTileContext(tile.TileContext):
    """TileContext that rewrites semaphore waits to fit this walrus build,
    which caps sync-waits per instruction very low (a matmul with 2 waits
    and a drain with 3 fail codegen). Excess waits are hoisted onto
    standalone same-engine InstEventSemaphore waits placed immediately
    before the owning instruction — semantically identical (the engine
    would stall at that point anyway)."""

    def _hoist_excess_waits(self):
        nc = self.nc
        assert self.sems is not None
        id_to_handle = {h.num: h for h in self.sems.allocated().values()}
        for bb in nc.main_func.blocks:
            orig = list(bb.instructions)
            if not any(
                getattr(ins, "sync_info", None) is not None
                and len(ins.sync_info.on_wait)
                > (0 if type(ins).__name__ == "InstMatmult" else 1)
                for ins in orig
            ):
                continue
            stolen_names = set()
            new_list = []
            for ins in orig:
                si = getattr(ins, "sync_info", None)
                waits = list(si.on_wait) if si is not None and si.on_wait else []
                keep_n = 0 if type(ins).__name__ == "InstMatmult" else 1
                if len(waits) > keep_n:
                    kept = []
                    emitted = []
                    for w in waits:
                        h = id_to_handle.get(w.id)
                        if (
                            h is None
                            or w.wait_mode != "sem-ge-imm"
                            or w.wait_reg is not None
                        ):
                            kept.append(w)
                        else:
                            emitted.append((h, w))
                    while emitted and len(kept) < keep_n:
                        kept.append(emitted.pop()[1])
                    si.on_wait = kept
                    for h, w in emitted:
                        # appends to the current bb; relocated via new_list
                        wi = nc.engines[ins.engine].wait_ge(h, w.wait_value)
                        stolen_names.add(wi.ins.name)
                        new_list.append(wi.ins)
                new_list.append(ins)
            # remove the side-effect-appended copies everywhere, then install
            # the rebuilt order for this block
            for bb2 in nc.main_func.blocks:
                if bb2.name == bb.name:
                    continue
                lst = list(bb2.instructions)
                filtered = [i for i in lst if i.name not in stolen_names]
                if len(filtered) != len(lst):
                    bb2.instructions = filtered
            bb.instructions = new_list

    def _drain_and_barrier(self, tick_clock, wait_clock):
        nc = self.nc
        self._hoist_excess_waits()
        probe = nc.sync.nop()
        wait_clock.add_sem_waits(
            probe.ins, ScopedClock({None: tick_clock.global_clock})
        )
        waits = list(probe.ins.sync_info.on_wait)
        probe.ins.sync_info.on_wait = []
        assert self.sems is not None
        id_to_handle = {h.num: h for h in self.sems.allocated().values()}
        for w in waits:
            h = id_to_handle.get(w.id)
            if h is None:
                probe.ins.sync_info.on_wait.append(w)
                continue
            nc.sync.wait_ge(h, w.wait_value)
        nc.sync.drain()
        nc.all_engine_barrier()
        popped = nc._tile_sem_poison_stack.pop()
        assert popped is self._sem_poison
        nc.clear_and_free_semaphores(list(self.sems.allocated().values()))
        nc.all_engine_barrier()


def _build_nc():
    nc = bass.Bass()
    x_d = nc.declare_dram_parameter(
        "x", [N_PER_CORE, C_IN, H, W], F32R, isOutput=False
    )
    w_d = nc.declare_dram_parameter("w", [C_IN, KH, KW, C_OUT], F32R, isOutput=False)
    o_d = nc.declare_dram_parameter(
        "out", [N_PER_CORE, C_OUT, HO, WO], F32, isOutput=True
    )

    Copy = mybir.ActivationFunctionType.Copy
    Sqrt = mybir.ActivationFunctionType.Sqrt
    Square = mybir.ActivationFunctionType.Square
    mult = mybir.AluOpType.mult
    add = mybir.AluOpType.add
    subtract = mybir.AluOpType.subtract

    with _SplitDrainTileContext(nc) as tc:
        with (
            tc.tile_pool(name="xp", bufs=1) as xp,
            tc.tile_pool(name="wp", bufs=1) as wp,
            tc.tile_pool(name="yp", bufs=BG + 1) as yp,
            tc.tile_pool(name="pp", bufs=8, space="PSUM") as pp,
            tc.tile_pool(name="sqp", bufs=2) as sqp,
            tc.tile_pool(name="accp", bufs=4) as accp,
            tc.tile_pool(name="stp", bufs=20) as stp,
        ):
            wt = wp.tile([C_IN, KH, KW, C_OUT], F32R)
            nc.sync.dma_start(wt[:], w_d[:])
            epsb = wp.tile([128, 1], F32, tag="eps")
            nc.vector.memset(epsb[:], 4.0 * EPS)

            for n in range(N_PER_CORE):
                xt = xp.tile([C_IN, H, W], F32R, tag="x")
                nc.sync.dma_start(xt[:], x_d[n])

                for c in range(2):
                    sums = accp.tile([128, N_RG], F32, tag="acc")
                    sqs = accp.tile([128, N_RG], F32, tag="acc")
                    yblocks = []
                    for bg in range(N_BG):
                        yb = yp.tile([128, BG, NCOL], F32, tag="y")
                        psums = [
                            pp.tile([128, NCOL], F32, tag="ps", name=f"ps{bg}_{j}")
                            for j in range(BG)
                        ]
                        for t in range(KH * KW):
                            kh, kw = divmod(t, KW)
                            lhs = wt[:, kh, kw, c * 128 : (c + 1) * 128]
                            for j in range(BG):
                                g = bg * BG + j
                                rhs = xt[
                                    :, RG * g + kh : RG * g + kh + RG, kw : kw + WO
                                ]
                                nc.tensor.matmul(
                                    psums[j][:],
                                    lhs,
                                    rhs,
                                    start=(t == 0),
                                    stop=(t == KH * KW - 1),
                                )
                        for j in range(BG):
                            g = bg * BG + j
                            nc.scalar.activation(
                                yb[:, j, :],
                                psums[j][:],
                                Copy,
                                accum_out=sums[:, g : g + 1],
                            )
                            sq = sqp.tile([128, NCOL], F32, tag="sq", name=f"sq{j}")
                            nc.scalar.activation(
                                sq[:],
                                psums[j][:],
                                Square,
                                accum_out=sqs[:, g : g + 1],
                            )
                        yblocks.append(yb)

                    # Per-(n, channel) stats over all 15876 pixels.
                    s1 = stp.tile([128, 1], F32, tag="st")
                    nc.vector.tensor_reduce(
                        s1[:], sums[:], axis=mybir.AxisListType.X, op=add
                    )
                    s2 = stp.tile([128, 1], F32, tag="st")
                    nc.vector.tensor_reduce(
                        s2[:], sqs[:], axis=mybir.AxisListType.X, op=add
                    )
                    mean = stp.tile([128, 1], F32, tag="st")
                    nc.vector.tensor_scalar_mul(mean[:], s1[:], 1.0 / PIX)
                    e2 = stp.tile([128, 1], F32, tag="st")
                    nc.vector.tensor_scalar_mul(e2[:], s2[:], 1.0 / PIX)
                    msq = stp.tile([128, 1], F32, tag="st")
                    nc.vector.tensor_mul(msq[:], mean[:], mean[:])
                    var = stp.tile([128, 1], F32, tag="st")
                    nc.vector.tensor_sub(var[:], e2[:], msq[:])
                    # alpha = rsqrt(var+eps)/2 = 1/sqrt(4*var + 4*eps)
                    std2 = stp.tile([128, 1], F32, tag="st")
                    nc.scalar.activation(
                        std2[:], var[:], Sqrt, bias=epsb[:], scale=4.0
                    )
                    alpha = stp.tile([128, 1], F32, tag="st")
                    nc.vector.reciprocal(alpha[:], std2[:])
                    malpha = stp.tile([128, 1], F32, tag="st")
                    nc.vector.tensor_mul(malpha[:], mean[:], alpha[:])

                    for bg, yb in enumerate(yblocks):
                        for j in range(BG):
                            nc.vector.tensor_scalar(
                                yb[:, j, :],
                                yb[:, j, :],
                                alpha[:],
                                malpha[:],
                                op0=mult,
                                op1=subtract,
                            )
                        nc.sync.dma_start(
                            o_d[
                                n,
                                c * 128 : (c + 1) * 128,
                                bg * BG * RG : (bg + 1) * BG * RG,
                                :,
                            ],
                            yb[:],
                        )
    return nc


_CACHED = None


def _get_exec():
    """Build the Bass program once and wrap it in a persistent jitted
    shard_map executor (mirrors bass2jax.run_bass_via_pjrt, but without
    donation so the callable can be re-invoked for timing)."""
    global _CACHED
    if _CACHED is not None:
        return _CACHED

    import jax
    from jax.experimental.shard_map import shard_map
    from jax.sharding import Mesh, PartitionSpec

    from concourse import bass2jax

    bass2jax.install_neuronx_cc_hook()
    nc = _build_nc()

    partition_name = (
        nc.partition_id_tensor.name if nc.partition_id_tensor else None
    )
    in_names = []
    out_names = []
    out_avals = []
    for alloc in nc.m.functions[0].allocations:
        if not isinstance(alloc, mybir.MemoryLocationSet):
            continue
        name = alloc.memorylocations[0].name
        if alloc.kind == "ExternalInput":
            if name != partition_name:
                in_names.append(name)
        elif alloc.kind == "ExternalOutput":
            out_names.append(name)
            out_avals.append(
                jax.core.ShapedArray(
                    tuple(alloc.tensor_shape), mybir.dt.np(alloc.dtype)
                )
            )
    n_params = len(in_names)
    all_in_names = in_names + out_names
    if partition_name is not None:
        all_in_names = all_in_names + [partition_name]

    def _body(*args):
        operands = list(args)
        if partition_name is not None:
            operands.append(bass2jax.partition_id_tensor())
        outs = bass2jax._bass_exec_p.bind(
            *operands,
            out_avals=tuple(out_avals),
            in_names=tuple(all_in_names),
            out_names=tuple(out_names),
            lowering_input_output_aliases=(),
            sim_require_finite=True,
            sim_require_nnan=True,
            nc=nc,
        )
        return tuple(outs)

    devices = jax.devices()[:N_CORES]
    mesh = Mesh(np.asarray(devices), ("core",))
    n_outs = len(out_names)
    sharded = jax.jit(
        shard_map(
            _body,
            mesh=mesh,
            in_specs=(PartitionSpec("core"),) * (n_params + n_outs),
            out_specs=(PartitionSpec("core"),) * n_outs,
            check_rep=False,
        ),
        keep_unused=True,
    )
    zeros = [
        np.zeros((N_CORES * a.shape[0], *a.shape[1:]), a.dtype) for a in out_avals
    ]
    _CACHED = (sharded, in_names, out_names, out_avals, zeros)
    return _CACHED


def _run(per_core_inputs):
    """per_core_inputs: dict name -> list of 8 per-core arrays.
    Returns dict name -> list of 8 per-core outputs."""
    sharded, in_names, out_names, out_avals, zeros = _get_exec()
    concat_in = [
        np.concatenate([np.asarray(per_core_inputs[nm][c]) for c in range(N_CORES)], axis=0)
        for nm in in_names
    ]
    out_arrs = sharded(*concat_in, *zeros)
    return {
        nm: np.asarray(out_arrs[i]).reshape(N_CORES, *out_avals[i].shape)
        for i, nm in enumerate(out_names)
    }


def kernel(x, weight, bias):
    x = np.ascontiguousarray(np.asarray(x, dtype=np.float32))
    weight = np.asarray(weight, dtype=np.float32)
    # bias is mathematically a no-op under InstanceNorm(affine=False).
    del bias
    # [C_out, C_in, KH, KW] -> [C_in, KH, KW, C_out] so each (kh, kw,
    # co-chunk) slice is a ready-to-use stationary operand.
    wt = np.ascontiguousarray(weight.transpose(1, 2, 3, 0))
    per_core = {
        "x": [x[c * N_PER_CORE : (c + 1) * N_PER_CORE] for c in range(N_CORES)],
        "w": [wt] * N_CORES,
    }
    outs = _run(per_core)["out"]  # [8, 2, 256, 126, 126]
    return outs.reshape(N, C_OUT, HO, WO)
